# revision 35
# baseline (speedup 1.0000x reference)
"""Trainium2 Bass kernel for nn_DiscreteLoss (data-parallel over batch).

Contract: kernel(**inputs) takes the FULL unsharded inputs (B=64) and
returns the FULL scalar loss.  Internally the batch dim is sharded over
8 NeuronCores (8 batches each); each core produces per-partition partial
sums for every loss term, which the host combines in float64.

Device-side strategy per core (8 batches, processed in 4 groups of 2):
  - the mapping gather along S is done on the TensorEngine: a one-hot
    matrix E_b[j, s] = (mapping[b, s] == j) is built on-chip (PE row
    broadcast of the mapping values + DVE is_equal against an iota
    column), then rz_g = E_b.T @ rzs etc.
  - the ground-truth subtraction is folded into the same PSUM
    accumulation group via a (-I) matmul.
  - squared-sum reduction of each PSUM result is a single ScalarE
    activation(Square, accum_out=...) pass; landmark / KL terms use one
    DVE pass each (tensor_tensor_reduce / scalar_tensor_tensor with
    accum_out).
"""

import contextlib
import ctypes
import os
import sys
import types

for _p in ("/opt/trn_rl_repo", "/root/.axon_site/_ro/trn_rl_repo"):
    if os.path.isdir(_p) and _p not in sys.path:
        sys.path.append(_p)

import numpy as np

# --- problem constants (hardcoded per spec) ---
B, S, N, D, V = 64, 128, 128, 512, 128
N_CORES = 8
BPC = B // N_CORES          # batches per core = 8
GROUPS = 4                  # groups of 2 batches
GB = 2                      # batches per group
ALPHA, BETA, GAMMA, EPS = 1.0, 0.1, 1.0, 1e-20
MARK = (0, 29, 88, 117)

_CACHE = {}


def _install_ntff_hook_shim():
    """run_bass_kernel_spmd(trace=True) looks for antenv.axon_hooks, which
    this image lacks; recreate the ctypes hook against libaxon_pjrt.so."""
    if "antenv.axon_hooks" in sys.modules:
        return
    so_path = "/opt/axon/libaxon_pjrt.so"

    def _get_hook():
        if not os.path.exists(so_path):
            return None
        lib = ctypes.CDLL(so_path)
        if not hasattr(lib, "axon_start_nrt_profile"):
            return None
        lib.axon_start_nrt_profile.argtypes = [
            ctypes.POINTER(ctypes.c_int64), ctypes.c_size_t]
        lib.axon_start_nrt_profile.restype = ctypes.c_int64
        lib.axon_stop_nrt_profile.argtypes = [ctypes.c_char_p]
        lib.axon_stop_nrt_profile.restype = ctypes.c_int64

        @contextlib.contextmanager
        def _hook(output_dir, device_ids):
            import jax
            jax.devices()
            if device_ids:
                ids = (ctypes.c_int64 * len(device_ids))(*device_ids)
                rc = lib.axon_start_nrt_profile(ids, len(device_ids))
            else:
                rc = lib.axon_start_nrt_profile(None, 0)
            if rc != 0:
                raise RuntimeError(f"axon_start_nrt_profile rc={rc}")
            try:
                yield
            finally:
                n = lib.axon_stop_nrt_profile(str(output_dir).encode())
                if n < 0:
                    raise RuntimeError(f"axon_stop_nrt_profile rc={n}")

        return _hook

    mod = types.ModuleType("antenv.axon_hooks")
    mod.get_axon_ntff_profile_hook = _get_hook
    mod.set_axon_ntff_profile_hook = lambda h: None
    sys.modules["antenv.axon_hooks"] = mod


def _build_program():
    import concourse.bacc as bacc
    import concourse.tile as tile
    from concourse import mybir

    f32 = mybir.dt.float32
    f32r = mybir.dt.float32r
    nc = bacc.Bacc(None, target_bir_lowering=False, debug=False)

    # ---- per-core DRAM parameters (host pre-transposed/concatenated) ----
    # rzzs[s, b, 0:512] = rzs, [512:1024] = zs
    d_rzzs = nc.declare_dram_parameter("rzzs", [S, BPC, 2 * D], f32r, isOutput=False)
    # pmg[s, b, 0:256]=pts, [256:512]=masks, [512:768]=pts_gt, [768:1024]=masks_gt
    d_pmg = nc.declare_dram_parameter("pmg", [S, BPC, 4 * 2 * N], f32r, isOutput=False)
    d_qy = nc.declare_dram_parameter("qy", [S, BPC, V], f32, isOutput=False)
    d_mapf = nc.declare_dram_parameter("mapf", [1, BPC * S], f32, isOutput=False)
    # all six small "best" tensors packed to one [128, 128] block
    d_best = nc.declare_dram_parameter("best_all", [128, 128], f32, isOutput=False)
    # host-built constants
    d_iota = nc.declare_dram_parameter("iota", [128, 1], f32, isOutput=False)
    d_wbest = nc.declare_dram_parameter("wbest", [128, 16], f32, isOutput=False)
    d_wslice = nc.declare_dram_parameter("wslice", [128, 16], f32, isOutput=False)
    # outputs: per-partition partial sums
    d_oauto = nc.declare_dram_parameter("o_auto", [128, GROUPS], f32, isOutput=True)
    d_odisk = nc.declare_dram_parameter("o_disk", [128, GROUPS], f32, isOutput=True)
    d_oseg = nc.declare_dram_parameter("o_seg", [128, GROUPS], f32, isOutput=True)
    d_oland = nc.declare_dram_parameter("o_land", [128, 4 * GROUPS], f32, isOutput=True)
    d_okld = nc.declare_dram_parameter("o_kld", [128, GROUPS], f32, isOutput=True)
    d_oqsum = nc.declare_dram_parameter("o_qsum", [128, GROUPS], f32, isOutput=True)
    d_obest = nc.declare_dram_parameter("o_best", [128, 4], f32, isOutput=True)

    SQUARE = mybir.ActivationFunctionType.Square
    LN = mybir.ActivationFunctionType.Ln
    COPY = mybir.ActivationFunctionType.Copy
    AL = mybir.AluOpType
    AX = mybir.AxisListType

    with tile.TileContext(nc) as tc:
        with contextlib.ExitStack() as ctx:
            singles = ctx.enter_context(tc.tile_pool(name="singles", bufs=1))
            pref = ctx.enter_context(tc.tile_pool(name="pref", bufs=1))
            data = ctx.enter_context(tc.tile_pool(name="data", bufs=2))
            junkp = ctx.enter_context(tc.tile_pool(name="junk", bufs=2))
            psp = ctx.enter_context(tc.tile_pool(name="ps", bufs=2, space="PSUM"))

            # ---- E-build inputs first: mapf gates the whole gather chain ----
            t_mapf = singles.tile([1, BPC * S], f32)
            nc.sync.dma_start(out=t_mapf[:], in_=d_mapf.ap())
            t_iota = singles.tile([128, 1], f32)
            nc.sync.dma_start(out=t_iota[:], in_=d_iota.ap())
            t_ones = singles.tile([1, 128], f32)
            nc.vector.memset(t_ones[:], 1.0)
            t_eps = singles.tile([128, 1], f32)
            nc.vector.memset(t_eps[:], EPS)

            # ---- prefetch ALL group data up front (each group has own slot) ----
            rzzs_t, pmg_t, qy_t = [], [], []
            for g in range(GROUPS):
                b0 = GB * g
                rzzs2 = pref.tile([128, GB, 2 * D], f32r, tag=f"rzzs{g}")
                nc.sync.dma_start(out=rzzs2[:], in_=d_rzzs.ap()[:, b0:b0 + GB, :])
                pmg2 = pref.tile([128, GB, 1024], f32r, tag=f"pmg{g}")
                nc.scalar.dma_start(out=pmg2[:], in_=d_pmg.ap()[:, b0:b0 + GB, :])
                qy2 = pref.tile([128, GB * V], f32, tag=f"qy{g}")
                nc.scalar.dma_start(out=qy2[:], in_=d_qy.ap()[:, b0:b0 + GB, :])
                rzzs_t.append(rzzs2); pmg_t.append(pmg2); qy_t.append(qy2)

            t_wbest = singles.tile([128, 16], f32)
            nc.sync.dma_start(out=t_wbest[:], in_=d_wbest.ap())
            t_wslice = singles.tile([128, 16], f32)
            nc.sync.dma_start(out=t_wslice[:], in_=d_wslice.ap())

            # accumulators (each column written exactly once)
            a_auto = singles.tile([128, GROUPS], f32)
            a_disk = singles.tile([128, GROUPS], f32)
            a_seg = singles.tile([128, GROUPS], f32)
            a_land = singles.tile([128, 4 * GROUPS], f32)
            a_kld = singles.tile([128, GROUPS], f32)
            a_qsum = singles.tile([128, GROUPS], f32)
            a_best = singles.tile([128, 4], f32)

            # ---- prologue: build all 8 one-hot matrices E_all[j, b*S+s] ----
            e_all = singles.tile([128, BPC * S], f32r)
            for h in range(2):
                ps_map = psp.tile([128, 512], f32, tag="rz")
                nc.tensor.matmul(
                    ps_map[:],
                    lhsT=t_ones[:],
                    rhs=t_mapf[:, h * 512:(h + 1) * 512],
                    start=True, stop=True,
                )
                nc.vector.tensor_scalar(
                    out=e_all[:, h * 512:(h + 1) * 512],
                    in0=ps_map[:],
                    scalar1=t_iota[:],
                    scalar2=None,
                    op0=AL.is_equal,
                )

            # ---- epilogue terms first so they overlap the main loop ----
            # best_all columns: [0:32]=best_rz [32:64]=logits [64:80]=best_pt
            # [80:96]=best_pt_gt [96:112]=best_mask [112:128]=best_mask_gt
            t_best = data.tile([128, 128], f32, tag="best")
            nc.scalar.dma_start(out=t_best[:], in_=d_best.ap())
            db = data.tile([128, 32], f32, tag="best32")
            nc.vector.tensor_sub(db[:], t_best[:, 0:32], t_best[:, 32:64])
            nc.scalar.activation(out=db[:], in_=db[:], func=SQUARE,
                                 accum_out=a_best[:, 0:1])
            dp = data.tile([128, 16], f32, tag="best16")
            nc.vector.tensor_sub(dp[:], t_best[:, 64:80], t_best[:, 80:96])
            nc.scalar.activation(out=dp[:], in_=dp[:], func=SQUARE,
                                 accum_out=a_best[:, 1:2])
            jb = junkp.tile([128, 16], f32, tag="jb")
            nc.vector.tensor_mul(jb[:], dp[:], t_wbest[:])
            nc.vector.tensor_reduce(out=a_best[:, 2:3], in_=jb[:],
                                    axis=AX.X, op=AL.add)
            dm = data.tile([128, 16], f32, tag="best16")
            nc.vector.tensor_sub(dm[:], t_best[:, 96:112], t_best[:, 112:128])
            nc.scalar.activation(out=dm[:], in_=dm[:], func=SQUARE)
            jb2 = junkp.tile([128, 16], f32, tag="jb")
            nc.vector.tensor_mul(jb2[:], dm[:], t_wslice[:])
            nc.vector.tensor_reduce(out=a_best[:, 3:4], in_=jb2[:],
                                    axis=AX.X, op=AL.add)
            nc.sync.dma_start(out=d_obest.ap(), in_=a_best[:])

            # ---- main loop: 4 groups x 2 batches (data already in flight) ----
            for g in range(GROUPS):
                b0 = GB * g
                rzzs2, pmg2, qy2 = rzzs_t[g], pmg_t[g], qy_t[g]

                ps_rz = psp.tile([128, GB, D], f32, tag="rz")
                ps_pm = psp.tile([128, GB, 512], f32, tag="pm")

                # gather on the TensorEngine (exact one-hot rows, f32r single-pass)
                for b2 in range(GB):
                    eb = e_all[:, (b0 + b2) * S:(b0 + b2 + 1) * S]
                    nc.tensor.matmul(ps_rz[:, b2, :], lhsT=eb,
                                     rhs=rzzs2[:, b2, 0:512],
                                     start=True, stop=True)
                    nc.tensor.matmul(ps_pm[:, b2, :], lhsT=eb,
                                     rhs=pmg2[:, b2, 0:512],
                                     start=True, stop=True)
                # subtract ground truth into SBUF (frees PSUM right after)
                d_rz = data.tile([128, GB, D], f32, tag="drz")
                nc.vector.tensor_sub(d_rz[:], ps_rz[:], rzzs2[:, :, 512:1024])
                d_pm = data.tile([128, GB, 512], f32, tag="dpm")
                nc.vector.tensor_sub(d_pm[:], ps_pm[:], pmg2[:, :, 512:1024])

                # squared sums (ScalarE): auto / disk / seg
                nc.scalar.activation(out=d_rz[:], in_=d_rz[:], func=SQUARE,
                                     accum_out=a_auto[:, g:g + 1])
                nc.scalar.activation(out=d_pm[:, :, 0:256], in_=d_pm[:, :, 0:256],
                                     func=SQUARE, accum_out=a_disk[:, g:g + 1])
                nc.scalar.activation(out=d_pm[:, :, 256:512], in_=d_pm[:, :, 256:512],
                                     func=SQUARE, accum_out=a_seg[:, g:g + 1])

                # landmark: 4 tiny strided reductions over the squared pts block
                for k, nk in enumerate(MARK):
                    nc.vector.tensor_reduce(
                        out=a_land[:, 4 * g + k:4 * g + k + 1],
                        in_=d_pm[:, :, 2 * nk:2 * nk + 2],
                        axis=AX.XY, op=AL.add,
                    )

                # KL pieces: sum qy*ln(qy+eps) and sum qy (host folds +lnV*sum qy)
                lnq = junkp.tile([128, GB * V], f32, tag="lnq")
                nc.scalar.activation(out=lnq[:], in_=qy2[:], func=LN, bias=t_eps[:])
                jkld = junkp.tile([128, GB * V], f32, tag="jk")
                nc.vector.tensor_mul(jkld[:], qy2[:], lnq[:])
                nc.vector.tensor_reduce(out=a_kld[:, g:g + 1], in_=jkld[:],
                                        axis=AX.X, op=AL.add)
                nc.scalar.activation(out=lnq[:], in_=qy2[:], func=COPY,
                                     accum_out=a_qsum[:, g:g + 1])

            # ---- store partials ----
            nc.sync.dma_start(out=d_oauto.ap(), in_=a_auto[:])
            nc.scalar.dma_start(out=d_odisk.ap(), in_=a_disk[:])
            nc.sync.dma_start(out=d_oseg.ap(), in_=a_seg[:])
            nc.scalar.dma_start(out=d_oland.ap(), in_=a_land[:])
            nc.sync.dma_start(out=d_okld.ap(), in_=a_kld[:])
            nc.scalar.dma_start(out=d_oqsum.ap(), in_=a_qsum[:])

    nc.compile()
    return nc


def _get_program():
    if "nc" not in _CACHE:
        _CACHE["nc"] = _build_program()
    return _CACHE["nc"]


def _host_constants():
    iota = np.arange(128, dtype=np.float32).reshape(128, 1)
    # wbest / wslice over the host-flattened [BPC*N*2] -> [128, 16] layout
    wbest = np.zeros(BPC * N * 2, dtype=np.float32)
    wslice = np.zeros(BPC * N * 2, dtype=np.float32)
    for b in range(BPC):
        for n in MARK:
            wbest[b * 2 * N + 2 * n] = 1.0
            wbest[b * 2 * N + 2 * n + 1] = 1.0
        wslice[b * 2 * N + 2 * 32: b * 2 * N + 2 * 96] = 1.0
    return {
        "iota": iota,
        "wbest": wbest.reshape(128, 16),
        "wslice": wslice.reshape(128, 16),
    }


def _shard_inputs(inputs):
    """Split the full B=64 inputs into 8 per-core input maps."""
    consts = _host_constants()
    f = lambda k: np.asarray(inputs[k], dtype=np.float32)
    # [B, S, X] views of everything, then one transpose+concat per pack
    rzzs = np.concatenate([f("rzs"), f("zs")], axis=2)                  # [B,S,1024]
    pmg = np.concatenate(
        [f("pts").reshape(B, S, 2 * N), f("masks").reshape(B, S, 2 * N),
         f("pts_gt").reshape(B, S, 2 * N), f("masks_gt").reshape(B, S, 2 * N)],
        axis=2)                                                          # [B,S,1024]
    qy = f("qy")
    mapf = np.asarray(inputs["mapping"]).astype(np.float32)
    best_all = np.concatenate(
        [f("best_rz").reshape(N_CORES, 128, 32),
         f("logits").reshape(N_CORES, 128, 32),
         f("best_pt").reshape(N_CORES, 128, 16),
         f("best_pt_gt").reshape(N_CORES, 128, 16),
         f("best_mask").reshape(N_CORES, 128, 16),
         f("best_mask_gt").reshape(N_CORES, 128, 16)],
        axis=2)                                                          # [8,128,128]

    in_maps = []
    for c in range(N_CORES):
        lo, hi = c * BPC, (c + 1) * BPC
        m = {
            "rzzs": np.ascontiguousarray(rzzs[lo:hi].transpose(1, 0, 2)),
            "pmg": np.ascontiguousarray(pmg[lo:hi].transpose(1, 0, 2)),
            "qy": np.ascontiguousarray(qy[lo:hi].transpose(1, 0, 2)),
            "mapf": np.ascontiguousarray(mapf[lo:hi].reshape(1, BPC * S)),
            "best_all": np.ascontiguousarray(best_all[c]),
        }
        m.update(consts)
        in_maps.append(m)
    return in_maps


def _combine(results, ln_v):
    """Host-side float64 reduction of the per-core partial sums."""
    s_auto = s_disk = s_seg = s_land = s_kld = s_qsum = 0.0
    s_best = np.zeros(4, dtype=np.float64)
    for r in results:
        s_auto += r["o_auto"].astype(np.float64).sum()
        s_disk += r["o_disk"].astype(np.float64).sum()
        s_seg += r["o_seg"].astype(np.float64).sum()
        s_land += r["o_land"].astype(np.float64).sum()
        s_kld += r["o_kld"].astype(np.float64).sum()
        s_qsum += r["o_qsum"].astype(np.float64).sum()
        s_best += r["o_best"].astype(np.float64).sum(axis=0)
    s_kld = s_kld + ln_v * s_qsum

    auto = s_auto / D
    disk = s_disk / (B * S)
    land = s_land / (B * S)
    seg = s_seg / (B * S * N * 2)
    kld = s_kld / (B * S)
    best_auto = s_best[0] / (B * D)
    best_disk = s_best[1] / (B * N * 2) / (B * N)
    best_land = s_best[2] / (B * N)
    best_seg = s_best[3] / (B * 64 * 2)

    reg = disk + land
    best_reg = best_disk + best_land
    ret = (GAMMA * (best_reg + best_auto + ALPHA * best_seg)
           + (reg + auto + ALPHA * seg)
           + BETA * kld)
    return np.float32(ret * B)


def run_sharded(inputs, trace=False):
    """Compile (cached), run on the 8 cores, return (scalar, BassKernelResults)."""
    _install_ntff_hook_shim()
    from concourse.bass_utils import run_bass_kernel_spmd

    ln_v = float(np.log(float(inputs["vector_dims"])))
    nc = _get_program()
    in_maps = _shard_inputs(inputs)
    res = run_bass_kernel_spmd(nc, in_maps, list(range(N_CORES)), trace=trace)
    return _combine(res.results, ln_v), res


def kernel(**inputs) -> np.ndarray:
    out, _ = run_sharded(inputs, trace=False)
    return out


# revision 36
# speedup vs baseline: 1.0854x; 1.0854x over previous
"""Trainium2 Bass kernel for nn_DiscreteLoss (data-parallel over batch).

Contract: kernel(**inputs) takes the FULL unsharded inputs (B=64) and
returns the FULL scalar loss.  Internally the batch dim is sharded over
8 NeuronCores (8 batches each); each core produces per-partition partial
sums for every loss term, which the host combines in float64.

Device-side strategy per core (8 batches, processed in 4 groups of 2):
  - the mapping gather along S is done on the TensorEngine: a one-hot
    matrix E_b[j, s] = (mapping[b, s] == j) is built on-chip (PE row
    broadcast of the mapping values + DVE is_equal against an iota
    column), then rz_g = E_b.T @ rzs etc.
  - the ground-truth subtraction is folded into the same PSUM
    accumulation group via a (-I) matmul.
  - squared-sum reduction of each PSUM result is a single ScalarE
    activation(Square, accum_out=...) pass; landmark / KL terms use one
    DVE pass each (tensor_tensor_reduce / scalar_tensor_tensor with
    accum_out).
"""

import contextlib
import ctypes
import os
import sys
import types

for _p in ("/opt/trn_rl_repo", "/root/.axon_site/_ro/trn_rl_repo"):
    if os.path.isdir(_p) and _p not in sys.path:
        sys.path.append(_p)

import numpy as np

# --- problem constants (hardcoded per spec) ---
B, S, N, D, V = 64, 128, 128, 512, 128
N_CORES = 8
BPC = B // N_CORES          # batches per core = 8
GROUPS = 4                  # groups of 2 batches
GB = 2                      # batches per group
ALPHA, BETA, GAMMA, EPS = 1.0, 0.1, 1.0, 1e-20
MARK = (0, 29, 88, 117)

_CACHE = {}


def _install_ntff_hook_shim():
    """run_bass_kernel_spmd(trace=True) looks for antenv.axon_hooks, which
    this image lacks; recreate the ctypes hook against libaxon_pjrt.so."""
    if "antenv.axon_hooks" in sys.modules:
        return
    so_path = "/opt/axon/libaxon_pjrt.so"

    def _get_hook():
        if not os.path.exists(so_path):
            return None
        lib = ctypes.CDLL(so_path)
        if not hasattr(lib, "axon_start_nrt_profile"):
            return None
        lib.axon_start_nrt_profile.argtypes = [
            ctypes.POINTER(ctypes.c_int64), ctypes.c_size_t]
        lib.axon_start_nrt_profile.restype = ctypes.c_int64
        lib.axon_stop_nrt_profile.argtypes = [ctypes.c_char_p]
        lib.axon_stop_nrt_profile.restype = ctypes.c_int64

        @contextlib.contextmanager
        def _hook(output_dir, device_ids):
            import jax
            jax.devices()
            if device_ids:
                ids = (ctypes.c_int64 * len(device_ids))(*device_ids)
                rc = lib.axon_start_nrt_profile(ids, len(device_ids))
            else:
                rc = lib.axon_start_nrt_profile(None, 0)
            if rc != 0:
                raise RuntimeError(f"axon_start_nrt_profile rc={rc}")
            try:
                yield
            finally:
                n = lib.axon_stop_nrt_profile(str(output_dir).encode())
                if n < 0:
                    raise RuntimeError(f"axon_stop_nrt_profile rc={n}")

        return _hook

    mod = types.ModuleType("antenv.axon_hooks")
    mod.get_axon_ntff_profile_hook = _get_hook
    mod.set_axon_ntff_profile_hook = lambda h: None
    sys.modules["antenv.axon_hooks"] = mod


def _build_program():
    import concourse.bacc as bacc
    import concourse.tile as tile
    from concourse import mybir

    f32 = mybir.dt.float32
    f32r = mybir.dt.float32r
    nc = bacc.Bacc(None, target_bir_lowering=False, debug=False)

    # ---- per-core DRAM parameters (host pre-transposed/concatenated) ----
    # rzzs[s, b, 0:512] = rzs, [512:1024] = zs
    d_rzzs = nc.declare_dram_parameter("rzzs", [S, BPC, 2 * D], f32r, isOutput=False)
    # pmg[s, b, 0:256]=pts, [256:512]=masks, [512:768]=pts_gt, [768:1024]=masks_gt
    d_pmg = nc.declare_dram_parameter("pmg", [S, BPC, 4 * 2 * N], f32r, isOutput=False)
    d_qy = nc.declare_dram_parameter("qy", [S, BPC, V], f32, isOutput=False)
    d_mapf = nc.declare_dram_parameter("mapf", [1, BPC * S], f32, isOutput=False)
    # all six small "best" tensors packed to one [128, 128] block
    d_best = nc.declare_dram_parameter("best_all", [128, 128], f32, isOutput=False)
    # host-built constants
    d_iota = nc.declare_dram_parameter("iota", [128, 1], f32, isOutput=False)
    d_wbest = nc.declare_dram_parameter("wbest", [128, 16], f32, isOutput=False)
    d_wslice = nc.declare_dram_parameter("wslice", [128, 16], f32, isOutput=False)
    # outputs: per-partition partial sums
    d_oauto = nc.declare_dram_parameter("o_auto", [128, GROUPS], f32, isOutput=True)
    d_odisk = nc.declare_dram_parameter("o_disk", [128, GROUPS], f32, isOutput=True)
    d_oland = nc.declare_dram_parameter("o_land", [128, 4 * GROUPS], f32, isOutput=True)
    d_okld = nc.declare_dram_parameter("o_kld", [128, GROUPS], f32, isOutput=True)
    d_oqsum = nc.declare_dram_parameter("o_qsum", [128, GROUPS], f32, isOutput=True)
    d_obest = nc.declare_dram_parameter("o_best", [128, 4], f32, isOutput=True)

    SQUARE = mybir.ActivationFunctionType.Square
    LN = mybir.ActivationFunctionType.Ln
    COPY = mybir.ActivationFunctionType.Copy
    AL = mybir.AluOpType
    AX = mybir.AxisListType

    with tile.TileContext(nc) as tc:
        with contextlib.ExitStack() as ctx:
            singles = ctx.enter_context(tc.tile_pool(name="singles", bufs=1))
            pref = ctx.enter_context(tc.tile_pool(name="pref", bufs=1))
            data = ctx.enter_context(tc.tile_pool(name="data", bufs=2))
            junkp = ctx.enter_context(tc.tile_pool(name="junk", bufs=2))
            psp = ctx.enter_context(tc.tile_pool(name="ps", bufs=2, space="PSUM"))

            # ---- E-build inputs first: mapf gates the whole gather chain ----
            t_mapf = singles.tile([1, BPC * S], f32)
            nc.sync.dma_start(out=t_mapf[:], in_=d_mapf.ap())
            t_iota = singles.tile([128, 1], f32)
            nc.sync.dma_start(out=t_iota[:], in_=d_iota.ap())
            t_ones = singles.tile([1, 128], f32)
            nc.vector.memset(t_ones[:], 1.0)
            t_eps = singles.tile([128, 1], f32)
            nc.vector.memset(t_eps[:], EPS)

            # ---- prefetch ALL group data up front (each group has own slot) ----
            rzzs_t, pmg_t, qy_t = [], [], []
            for g in range(GROUPS):
                b0 = GB * g
                rzzs2 = pref.tile([128, GB, 2 * D], f32r, tag=f"rzzs{g}")
                nc.sync.dma_start(out=rzzs2[:], in_=d_rzzs.ap()[:, b0:b0 + GB, :])
                pmg2 = pref.tile([128, GB, 1024], f32r, tag=f"pmg{g}")
                nc.scalar.dma_start(out=pmg2[:], in_=d_pmg.ap()[:, b0:b0 + GB, :])
                qy2 = pref.tile([128, GB * V], f32, tag=f"qy{g}")
                nc.scalar.dma_start(out=qy2[:], in_=d_qy.ap()[:, b0:b0 + GB, :])
                rzzs_t.append(rzzs2); pmg_t.append(pmg2); qy_t.append(qy2)

            t_wbest = singles.tile([128, 16], f32)
            nc.sync.dma_start(out=t_wbest[:], in_=d_wbest.ap())
            t_wslice = singles.tile([128, 16], f32)
            nc.sync.dma_start(out=t_wslice[:], in_=d_wslice.ap())

            # accumulators (each column written exactly once)
            a_auto = singles.tile([128, GROUPS], f32)
            a_disk = singles.tile([128, GROUPS], f32)
            a_land = singles.tile([128, 4 * GROUPS], f32)
            a_kld = singles.tile([128, GROUPS], f32)
            a_qsum = singles.tile([128, GROUPS], f32)
            a_best = singles.tile([128, 4], f32)

            # ---- prologue: build all 8 one-hot matrices E_all[j, b*S+s] ----
            e_all = singles.tile([128, BPC * S], f32r)
            for h in range(2):
                ps_map = psp.tile([128, 512], f32, tag="rz")
                nc.tensor.matmul(
                    ps_map[:],
                    lhsT=t_ones[:],
                    rhs=t_mapf[:, h * 512:(h + 1) * 512],
                    start=True, stop=True,
                )
                nc.vector.tensor_scalar(
                    out=e_all[:, h * 512:(h + 1) * 512],
                    in0=ps_map[:],
                    scalar1=t_iota[:],
                    scalar2=None,
                    op0=AL.is_equal,
                )

            # ---- epilogue terms first so they overlap the main loop ----
            # best_all columns: [0:32]=best_rz [32:64]=logits [64:80]=best_pt
            # [80:96]=best_pt_gt [96:112]=best_mask [112:128]=best_mask_gt
            t_best = data.tile([128, 128], f32, tag="best")
            nc.scalar.dma_start(out=t_best[:], in_=d_best.ap())
            db = data.tile([128, 32], f32, tag="best32")
            nc.vector.tensor_sub(db[:], t_best[:, 0:32], t_best[:, 32:64])
            nc.scalar.activation(out=db[:], in_=db[:], func=SQUARE,
                                 accum_out=a_best[:, 0:1])
            dp = data.tile([128, 16], f32, tag="best16")
            nc.vector.tensor_sub(dp[:], t_best[:, 64:80], t_best[:, 80:96])
            nc.scalar.activation(out=dp[:], in_=dp[:], func=SQUARE,
                                 accum_out=a_best[:, 1:2])
            jb = junkp.tile([128, 16], f32, tag="jb")
            nc.vector.tensor_mul(jb[:], dp[:], t_wbest[:])
            nc.vector.tensor_reduce(out=a_best[:, 2:3], in_=jb[:],
                                    axis=AX.X, op=AL.add)
            dm = data.tile([128, 16], f32, tag="best16")
            nc.vector.tensor_sub(dm[:], t_best[:, 96:112], t_best[:, 112:128])
            nc.scalar.activation(out=dm[:], in_=dm[:], func=SQUARE)
            jb2 = junkp.tile([128, 16], f32, tag="jb")
            nc.vector.tensor_mul(jb2[:], dm[:], t_wslice[:])
            nc.vector.tensor_reduce(out=a_best[:, 3:4], in_=jb2[:],
                                    axis=AX.X, op=AL.add)
            nc.sync.dma_start(out=d_obest.ap(), in_=a_best[:])

            # ---- main loop: 4 groups x 2 batches (data already in flight) ----
            for g in range(GROUPS):
                b0 = GB * g
                rzzs2, pmg2, qy2 = rzzs_t[g], pmg_t[g], qy_t[g]

                ps_rz = psp.tile([128, GB, D], f32, tag="rz")
                ps_pm = psp.tile([128, GB, 512], f32, tag="pm")

                # gather on the TensorEngine (exact one-hot rows, f32r single-pass)
                for b2 in range(GB):
                    eb = e_all[:, (b0 + b2) * S:(b0 + b2 + 1) * S]
                    nc.tensor.matmul(ps_rz[:, b2, :], lhsT=eb,
                                     rhs=rzzs2[:, b2, 0:512],
                                     start=True, stop=True)
                    nc.tensor.matmul(ps_pm[:, b2, :], lhsT=eb,
                                     rhs=pmg2[:, b2, 0:512],
                                     start=True, stop=True)
                # subtract ground truth into SBUF (frees PSUM right after)
                d_rz = data.tile([128, GB, D], f32, tag="drz")
                nc.vector.tensor_sub(d_rz[:], ps_rz[:], rzzs2[:, :, 512:1024])
                d_pm = data.tile([128, GB, 512], f32, tag="dpm")
                nc.vector.tensor_sub(d_pm[:], ps_pm[:], pmg2[:, :, 512:1024])

                # squared sums (ScalarE): auto / disk / seg
                nc.scalar.activation(out=d_rz[:], in_=d_rz[:], func=SQUARE,
                                     accum_out=a_auto[:, g:g + 1])
                nc.scalar.activation(out=d_pm[:], in_=d_pm[:], func=SQUARE,
                                     accum_out=a_disk[:, g:g + 1])

                # landmark: 4 tiny strided reductions over the squared pts block
                for k, nk in enumerate(MARK):
                    nc.vector.tensor_reduce(
                        out=a_land[:, 4 * g + k:4 * g + k + 1],
                        in_=d_pm[:, :, 2 * nk:2 * nk + 2],
                        axis=AX.XY, op=AL.add,
                    )

                # KL pieces: sum qy*ln(qy+eps) and sum qy (host folds +lnV*sum qy)
                lnq = junkp.tile([128, GB * V], f32, tag="lnq")
                nc.scalar.activation(out=lnq[:], in_=qy2[:], func=LN, bias=t_eps[:])
                jkld = junkp.tile([128, GB * V], f32, tag="jk")
                nc.vector.tensor_mul(jkld[:], qy2[:], lnq[:])
                nc.vector.tensor_reduce(out=a_kld[:, g:g + 1], in_=jkld[:],
                                        axis=AX.X, op=AL.add)
                nc.vector.tensor_reduce(out=a_qsum[:, g:g + 1], in_=qy2[:],
                                        axis=AX.X, op=AL.add)

            # ---- store partials ----
            nc.sync.dma_start(out=d_oauto.ap(), in_=a_auto[:])
            nc.scalar.dma_start(out=d_odisk.ap(), in_=a_disk[:])
            nc.scalar.dma_start(out=d_oland.ap(), in_=a_land[:])
            nc.sync.dma_start(out=d_okld.ap(), in_=a_kld[:])
            nc.scalar.dma_start(out=d_oqsum.ap(), in_=a_qsum[:])

    nc.compile()
    return nc


def _get_program():
    if "nc" not in _CACHE:
        _CACHE["nc"] = _build_program()
    return _CACHE["nc"]


def _host_constants():
    iota = np.arange(128, dtype=np.float32).reshape(128, 1)
    # wbest / wslice over the host-flattened [BPC*N*2] -> [128, 16] layout
    wbest = np.zeros(BPC * N * 2, dtype=np.float32)
    wslice = np.zeros(BPC * N * 2, dtype=np.float32)
    for b in range(BPC):
        for n in MARK:
            wbest[b * 2 * N + 2 * n] = 1.0
            wbest[b * 2 * N + 2 * n + 1] = 1.0
        wslice[b * 2 * N + 2 * 32: b * 2 * N + 2 * 96] = 1.0
    return {
        "iota": iota,
        "wbest": wbest.reshape(128, 16),
        "wslice": wslice.reshape(128, 16),
    }


def _shard_inputs(inputs):
    """Split the full B=64 inputs into 8 per-core input maps."""
    consts = _host_constants()
    f = lambda k: np.asarray(inputs[k], dtype=np.float32)
    # [B, S, X] views of everything, then one transpose+concat per pack
    rzzs = np.concatenate([f("rzs"), f("zs")], axis=2)                  # [B,S,1024]
    # masks are pre-scaled by sqrt(1/(2N)) = 1/16 (exact in fp32) so the
    # seg sum folds into the disk accumulator with the right normalizer
    msc = np.float32(1.0 / 16.0)
    pmg = np.concatenate(
        [f("pts").reshape(B, S, 2 * N), f("masks").reshape(B, S, 2 * N) * msc,
         f("pts_gt").reshape(B, S, 2 * N), f("masks_gt").reshape(B, S, 2 * N) * msc,
         ], axis=2)                                                      # [B,S,1024]
    qy = f("qy")
    mapf = np.asarray(inputs["mapping"]).astype(np.float32)
    best_all = np.concatenate(
        [f("best_rz").reshape(N_CORES, 128, 32),
         f("logits").reshape(N_CORES, 128, 32),
         f("best_pt").reshape(N_CORES, 128, 16),
         f("best_pt_gt").reshape(N_CORES, 128, 16),
         f("best_mask").reshape(N_CORES, 128, 16),
         f("best_mask_gt").reshape(N_CORES, 128, 16)],
        axis=2)                                                          # [8,128,128]

    in_maps = []
    for c in range(N_CORES):
        lo, hi = c * BPC, (c + 1) * BPC
        m = {
            "rzzs": np.ascontiguousarray(rzzs[lo:hi].transpose(1, 0, 2)),
            "pmg": np.ascontiguousarray(pmg[lo:hi].transpose(1, 0, 2)),
            "qy": np.ascontiguousarray(qy[lo:hi].transpose(1, 0, 2)),
            "mapf": np.ascontiguousarray(mapf[lo:hi].reshape(1, BPC * S)),
            "best_all": np.ascontiguousarray(best_all[c]),
        }
        m.update(consts)
        in_maps.append(m)
    return in_maps


def _combine(results, ln_v):
    """Host-side float64 reduction of the per-core partial sums."""
    s_auto = s_disk = s_land = s_kld = s_qsum = 0.0
    s_best = np.zeros(4, dtype=np.float64)
    for r in results:
        s_auto += r["o_auto"].astype(np.float64).sum()
        s_disk += r["o_disk"].astype(np.float64).sum()
        s_land += r["o_land"].astype(np.float64).sum()
        s_kld += r["o_kld"].astype(np.float64).sum()
        s_qsum += r["o_qsum"].astype(np.float64).sum()
        s_best += r["o_best"].astype(np.float64).sum(axis=0)
    s_kld = s_kld + ln_v * s_qsum

    auto = s_auto / D
    disk_plus_seg = s_disk / (B * S)   # includes ALPHA*seg via the 1/16 mask pre-scale
    land = s_land / (B * S)
    kld = s_kld / (B * S)
    best_auto = s_best[0] / (B * D)
    best_disk = s_best[1] / (B * N * 2) / (B * N)
    best_land = s_best[2] / (B * N)
    best_seg = s_best[3] / (B * 64 * 2)

    best_reg = best_disk + best_land
    ret = (GAMMA * (best_reg + best_auto + ALPHA * best_seg)
           + (disk_plus_seg + land + auto)
           + BETA * kld)
    return np.float32(ret * B)


def run_sharded(inputs, trace=False):
    """Compile (cached), run on the 8 cores, return (scalar, BassKernelResults)."""
    _install_ntff_hook_shim()
    from concourse.bass_utils import run_bass_kernel_spmd

    ln_v = float(np.log(float(inputs["vector_dims"])))
    nc = _get_program()
    in_maps = _shard_inputs(inputs)
    res = run_bass_kernel_spmd(nc, in_maps, list(range(N_CORES)), trace=trace)
    return _combine(res.results, ln_v), res


def kernel(**inputs) -> np.ndarray:
    out, _ = run_sharded(inputs, trace=False)
    return out
